# revision 21
# baseline (speedup 1.0000x reference)
"""Trainium2 Bass kernel for a dense graph-transformer block.

Reference computation (per batch item b, with C=256, N=H*W=1024):
    nodes = x[b].reshape(C, N).T                      # [N, C]
    q     = nodes @ proj_w.T + proj_b                 # [N, C]
    S     = (q @ q.T) / sqrt(C)                       # [N, N]  (symmetric!)
    A     = softmax(S, axis=-1)
    agg   = A @ nodes                                 # [N, C]
    h     = gelu(agg @ w1.T + b1)  (erf gelu)
    out   = h @ w2.T + b2
    y[b]  = x[b] + out.T.reshape(C, H, W)

Kernel strategy (data-parallel over batch, 2 items per core, 8 cores):
  Everything is kept in the "transposed" layout [C-on-partitions, N-free],
  which is the *natural* layout of x[b] in HBM.  Matmul outputs land in this
  layout automatically (out partition dim = stationary's free dim).

  -  qT = 0.25*(proj_w @ X) + 0.25*proj_b    (so S = qT.T@qT directly)
  -  S is symmetric, and its entries are small (|S| <~ 7), so softmax is
     computed WITHOUT max subtraction: E = exp(S) is then also symmetric,
     which lets E's stored tiles serve as both lhs and rhs views.
  -  Row sums Z come for free from the ACT accumulator during the exp pass.
  -  aggT_unnorm = nodes.T-weighted sum:  matmul(lhsT=XT, rhs=E)
     then scaled by (1/Z)[n] broadcast along partitions.
  -  MLP stays in T-layout: biases are per-partition, gelu fuses with the
     PSUM->SBUF copy on the scalar engine.
  -  Residual add fuses with b2-add in one DVE scalar_tensor_tensor op.

  Matmul operands are bitcast to float32r (fp32 bits, fast PE mode:
  1 cycle/row when moving free dim >= 256, vs 4 for plain fp32).
"""

import os
import sys

import numpy as np

for _p in ("/opt/trn_rl_repo", "/root/.axon_site/_ro/trn_rl_repo"):
    if os.path.isdir(_p) and _p not in sys.path:
        sys.path.insert(0, _p)

import concourse.bass as bass
import concourse.bacc as bacc
import concourse.mybir as mybir
from concourse import tile
from concourse.alu_op_type import AluOpType
from concourse.bass_utils import run_bass_kernel_spmd

F32 = mybir.dt.float32
F32R = mybir.dt.float32r
AFT = mybir.ActivationFunctionType

C = 256          # channels
N = 1024         # nodes = H*W
CT = C // 128    # channel partition-tiles (2)
NT = N // 128    # node partition-tiles (8)
NF = N // 512    # node free-chunks of 512 (2)
N_CORES = 8
ITEMS = 2        # batch items per core (B=16 / 8 cores)


def ts(i, size):
    return slice(i * size, (i + 1) * size)


def _r(ap):
    """bitcast an AP to float32r for fast PE consumption"""
    return ap.bitcast(F32R)


def build_nc(gelu_func=AFT.Gelu):
    nc = bacc.Bacc(None, target_bir_lowering=False)

    xs_d = nc.dram_tensor("xs", [ITEMS, C, N], F32R, kind="ExternalInput")
    pwT_d = nc.dram_tensor("pwT", [C, C], F32R, kind="ExternalInput")
    w1T_d = nc.dram_tensor("w1T", [C, C], F32R, kind="ExternalInput")
    w2T_d = nc.dram_tensor("w2T", [C, C], F32R, kind="ExternalInput")
    ones_d = nc.dram_tensor("ones", [1, 128], F32R, kind="ExternalInput")
    onesc_d = nc.dram_tensor("onesc", [128, 1], F32R, kind="ExternalInput")
    pb_d = nc.dram_tensor("pb", [128, CT], F32, kind="ExternalInput")
    b1_d = nc.dram_tensor("b1", [128, CT], F32, kind="ExternalInput")
    b2_d = nc.dram_tensor("b2", [128, CT], F32, kind="ExternalInput")
    ident_d = nc.dram_tensor("ident", [128, 128], F32, kind="ExternalInput")
    y_d = nc.dram_tensor("y", [ITEMS, C, N], F32, kind="ExternalOutput")

    with tile.TileContext(nc) as tc:
        with (
            tc.tile_pool(name="const", bufs=1) as constp,
            tc.tile_pool(name="xin", bufs=2) as xp,
            tc.tile_pool(name="qt", bufs=2) as qp,
            tc.tile_pool(name="ebig", bufs=1) as ep,
            tc.tile_pool(name="xtp", bufs=2) as xtp,
            tc.tile_pool(name="aggp", bufs=2) as aggp,
            tc.tile_pool(name="htp", bufs=2) as hp,
            tc.tile_pool(name="yp", bufs=2) as yp,
            tc.tile_pool(name="statp", bufs=2) as statp,
            tc.tile_pool(name="psmm", bufs=3, space=bass.MemorySpace.PSUM) as psmm,
            tc.tile_pool(name="pstr", bufs=2, space=bass.MemorySpace.PSUM) as pstr,
            tc.tile_pool(name="psz", bufs=2, space=bass.MemorySpace.PSUM) as pszp,
            tc.tile_pool(name="psbc", bufs=1, space=bass.MemorySpace.PSUM) as psbc,
        ):
            # ---- constants ----
            # PE instructions tolerate only ONE sync wait, so every tile the
            # tensor engine reads is staged through a single engine (ACT):
            # PE then only ever waits on the ACT (or DVE) semaphore.
            pwT_r = constp.tile([128, CT, C], F32R)
            w1T_r = constp.tile([128, CT, C], F32R)
            w2T_r = constp.tile([128, CT, C], F32R)
            pwT = constp.tile([128, CT, C], F32R)
            w1T = constp.tile([128, CT, C], F32R)
            w2T = constp.tile([128, CT, C], F32R)
            for t_sb, t_d in ((pwT_r, pwT_d), (w1T_r, w1T_d), (w2T_r, w2T_d)):
                nc.sync.dma_start(
                    t_sb[:], t_d.ap().rearrange("(t p) m -> p t m", p=128)
                )
            pb = constp.tile([128, CT], F32)
            b1 = constp.tile([128, CT], F32)
            b2 = constp.tile([128, CT], F32)
            ident_r = constp.tile([128, 128], F32)
            ident = constp.tile([128, 128], F32)
            ones_r = constp.tile([1, 128], F32R)
            ones = constp.tile([1, 128], F32R)
            onesc_r = constp.tile([128, 1], F32R)
            onesc = constp.tile([128, 1], F32R)
            nc.sync.dma_start(ones_r[:], ones_d.ap())
            nc.sync.dma_start(onesc_r[:], onesc_d.ap())
            nc.sync.dma_start(pb[:], pb_d.ap())
            nc.sync.dma_start(b1[:], b1_d.ap())
            nc.sync.dma_start(b2[:], b2_d.ap())
            nc.sync.dma_start(ident_r[:], ident_d.ap())
            for dst, srcp in ((pwT, pwT_r), (w1T, w1T_r), (w2T, w2T_r),
                              (ident, ident_r), (ones, ones_r), (onesc, onesc_r)):
                nc.scalar.copy(dst[:], srcp[:])

            for it in range(ITEMS):
                xv = xs_d.ap()[it].rearrange("(t p) n -> p t n", p=128)
                yv = y_d.ap()[it].rearrange("(t p) n -> p t n", p=128)

                Xr = xp.tile([128, CT, N], F32R, tag="Xr")
                X = xp.tile([128, CT, N], F32R, tag="X")
                for nf in range(NF):
                    for ct in range(CT):
                        nc.sync.dma_start(
                            Xr[:, ct, ts(nf, 512)], xv[:, ct, ts(nf, 512)]
                        )
                        nc.scalar.copy(
                            X[:, ct, ts(nf, 512)], Xr[:, ct, ts(nf, 512)]
                        )

                # ---- qT = 0.25*(proj_w @ X) + 0.25*proj_b  -> [c_p, n] ----
                qT = qp.tile([128, CT, N], F32R, tag="qT")
                for mt in range(CT):
                    for nf in range(NF):
                        ps = psmm.tile([128, 512], F32, tag="mm")
                        for kt in range(CT):
                            nc.tensor.matmul(
                                ps[:],
                                _r(pwT[:, kt, ts(mt, 128)]),
                                _r(X[:, kt, ts(nf, 512)]),
                                start=(kt == 0),
                                stop=(kt == CT - 1),
                            )
                        nc.scalar.activation(
                            qT[:, mt, ts(nf, 512)],
                            ps[:],
                            AFT.Identity,
                            bias=pb[:, mt : mt + 1],
                            scale=0.25,
                        )

                # ---- S = qT.T @ qT ;  E = exp(S) ----
                # Z[n] (softmax denominators) = column sums of E (E symmetric),
                # accumulated as rank-reducing ones-matmuls into [1, 512] rows.
                E = ep.tile([128, NT, N], F32R, tag="E")
                pszs = [pszp.tile([1, 512], F32, tag="psz", name=f"psz{it}_{i}") for i in range(NF)]
                for nt in range(NT):
                    for mf in range(NF):
                        ps = psmm.tile([128, 512], F32, tag="mm")
                        for kt in range(CT):
                            nc.tensor.matmul(
                                ps[:],
                                _r(qT[:, kt, ts(nt, 128)]),
                                _r(qT[:, kt, ts(mf, 512)]),
                                start=(kt == 0),
                                stop=(kt == CT - 1),
                            )
                        nc.scalar.activation(
                            E[:, nt, ts(mf, 512)],
                            ps[:],
                            AFT.Exp,
                        )
                for mf in range(NF):
                    for nt in range(NT):
                        nc.tensor.matmul(
                            pszs[mf][:],
                            onesc[:, 0:1],
                            E[:, nt, ts(mf, 512)],
                            start=(nt == 0),
                            stop=(nt == NT - 1),
                        )

                # ---- rrow = 1/Z as a [1, N] row ----
                rrow = statp.tile([1, N], F32R, tag="rrow")
                with nc.allow_low_precision(reason="f32r rounding of 1/Z is ~fp32"):
                    for nf in range(NF):
                        nc.vector.reciprocal(rrow[0:1, ts(nf, 512)], pszs[nf][0:1, :])

                # ---- XT = nodes [n_p, c] via PE transposes ----
                XT = xtp.tile([128, NT, C], F32R, tag="XT")
                for nt in range(NT):
                    for ct in range(CT):
                        pt = pstr.tile([128, 128], F32, tag="tr")
                        nc.tensor.transpose(pt[:], X[:, ct, ts(nt, 128)].bitcast(F32), ident[:])
                        nc.vector.tensor_copy(XT[:, nt, ts(ct, 128)], pt[:])

                # ---- aggT = (XT.T @ E) * (1/Z)[n-broadcast] ----
                aggT = aggp.tile([128, CT, N], F32R, tag="aggT")
                for nf in range(NF):
                    Rbc = psbc.tile([128, 512], F32, tag="Rbc")
                    nc.tensor.matmul(
                        Rbc[:],
                        ones[0:1, :],
                        rrow[0:1, ts(nf, 512)],
                        start=True,
                        stop=True,
                    )
                    Rbs = statp.tile([128, 512], F32, tag="Rbs")
                    nc.vector.tensor_copy(Rbs[:], Rbc[:])
                    for ct in range(CT):
                        ps = psmm.tile([128, 512], F32, tag="mm")
                        for mt in range(NT):
                            nc.tensor.matmul(
                                ps[:],
                                _r(XT[:, mt, ts(ct, 128)]),
                                _r(E[:, mt, ts(nf, 512)]),
                                start=(mt == 0),
                                stop=(mt == NT - 1),
                            )
                        nc.vector.tensor_tensor(
                            aggT[:, ct, ts(nf, 512)],
                            ps[:],
                            Rbs[:],
                            AluOpType.mult,
                        )

                # ---- hT = gelu(w1 @ aggT + b1) ----
                hT = hp.tile([128, CT, N], F32R, tag="hT")
                for mt in range(CT):
                    for nf in range(NF):
                        ps = psmm.tile([128, 512], F32, tag="mm")
                        for kt in range(CT):
                            nc.tensor.matmul(
                                ps[:],
                                _r(w1T[:, kt, ts(mt, 128)]),
                                _r(aggT[:, kt, ts(nf, 512)]),
                                start=(kt == 0),
                                stop=(kt == CT - 1),
                            )
                        nc.scalar.activation(
                            hT[:, mt, ts(nf, 512)],
                            ps[:],
                            gelu_func,
                            bias=b1[:, mt : mt + 1],
                        )

                # ---- y = X + (w2 @ hT + b2) ----
                Y = yp.tile([128, CT, N], F32, tag="Y")
                for mt in range(CT):
                    for nf in range(NF):
                        ps = psmm.tile([128, 512], F32, tag="mm")
                        for kt in range(CT):
                            nc.tensor.matmul(
                                ps[:],
                                _r(w2T[:, kt, ts(mt, 128)]),
                                _r(hT[:, kt, ts(nf, 512)]),
                                start=(kt == 0),
                                stop=(kt == CT - 1),
                            )
                        nc.vector.scalar_tensor_tensor(
                            Y[:, mt, ts(nf, 512)],
                            ps[:],
                            b2[:, mt : mt + 1],
                            X[:, mt, ts(nf, 512)].bitcast(F32),
                            AluOpType.add,
                            AluOpType.add,
                        )
                for ct in range(CT):
                    nc.sync.dma_start(yv[:, ct, :], Y[:, ct, :])

    nc.compile()
    return nc


_NC_CACHE = {}


def _get_nc():
    if "nc" not in _NC_CACHE:
        _NC_CACHE["nc"] = build_nc()
    return _NC_CACHE["nc"]


def make_in_maps(x, proj_w, proj_b, w1, b1, w2, b2):
    B = x.shape[0]
    xs = np.ascontiguousarray(x.reshape(B, C, N).astype(np.float32))
    shared = {
        "pwT": np.ascontiguousarray(proj_w.T.astype(np.float32)),
        "w1T": np.ascontiguousarray(w1.T.astype(np.float32)),
        "w2T": np.ascontiguousarray(w2.T.astype(np.float32)),
        "pb": np.ascontiguousarray((0.25 * proj_b).reshape(CT, 128).T.astype(np.float32)),
        "b1": np.ascontiguousarray(b1.reshape(CT, 128).T.astype(np.float32)),
        "b2": np.ascontiguousarray(b2.reshape(CT, 128).T.astype(np.float32)),
        "ident": np.eye(128, dtype=np.float32),
        "ones": np.ones((1, 128), dtype=np.float32),
        "onesc": np.ones((128, 1), dtype=np.float32),
    }
    in_maps = []
    for c in range(N_CORES):
        m = dict(shared)
        m["xs"] = np.ascontiguousarray(xs[c * ITEMS : (c + 1) * ITEMS])
        in_maps.append(m)
    return in_maps


def kernel(x, proj_w, proj_b, w1, b1, w2, b2, _trace=False, **trace_kw):
    nc = _get_nc()
    in_maps = make_in_maps(x, proj_w, proj_b, w1, b1, w2, b2)
    res = run_bass_kernel_spmd(
        nc, in_maps, list(range(N_CORES)), trace=_trace, **trace_kw
    )
    outs = [r["y"] for r in res.results]
    B, _, H, W = x.shape
    y = np.concatenate(outs, axis=0).reshape(B, C, H, W).astype(np.float32)
    if _trace:
        kernel.last_result = res
    return y
